# revision 3
# baseline (speedup 1.0000x reference)
"""Trainium2 Bass kernel for nn_NeuralTensorDiagLayer.

Computes out = tanh(concat([e1, e2], -1) @ V + diag + b) where
diag[k] = (sum_b(e1*e2) @ W[k]) / (B*D), broadcast over batch.

Sharding (8 NeuronCores, 2D: 4 batch groups x 2 k_out halves):
  - Core c handles batch rows [1024*(c//2), 1024*(c//2+1)) and k_out
    columns [1024*(c%2), 1024*(c%2+1)).
  - All main-path tensors are cast to bf16 on the host (V from
    uniform(-1,1), x = concat(e1,e2) transposed): rel-err budget is 2e-2
    and bf16 end-to-end measures ~1e-2, while halving HBM traffic and
    keeping the TensorEngine at 1 col/cycle.
  - x^T and V stream into SBUF fully resident via interleaved [128,1024]
    DMAs (2 KiB lines) ordered so contraction tile j (x1_j, x2_j, v_j,
    v_{16+j}) lands early; the main matmul's first PSUM group chases the
    DMA stream and the rest runs from SBUF at full rate.
  - Main matmul: 3 PSUM groups of (3,3,2) k-tiles x 2 batch-half banks.
    Groups 0/1 drain PSUM->stage with DVE/ScalarE copies split per bank;
    group 2 is tanh'ed directly out of PSUM (ScalarE reads PSUM).
  - diag: per-tile fused mul+reduce partials on DVE as x tiles arrive,
    8-core AllReduce of s=[128,16] (each batch row counted twice -> 0.5
    folded into DIAG_SCALE), then a 256-col diag slice as 16 f32r
    matmuls (N=256 -> 1 cycle/row) pinned AFTER main group 1 in the
    TensorE stream (AllReduce is long done by then; pinning avoids the
    baseline's 17us TensorE stall), AllGather over [[0,2,4,6],[1,3,5,7]]
    assembles each k_out half (diag slice index sc = (c%2)*4 + c//2 is
    applied host-side so the device program stays SPMD-identical).
  - tanh+bias on ScalarE with diag as per-partition bias, fp32 out tiles
    DMA'd per k-tile ([k_out, batch] transposed); host reassembles.
"""

import os
import sys

for _p in ("/opt/trn_rl_repo", "/root/.axon_site/_ro/trn_rl_repo"):
    if os.path.isdir(_p) and _p not in sys.path:
        sys.path.append(_p)

import numpy as np

N_CORES = 8
B, D, K_OUT = 4096, 2048, 2048
FEAT = 2 * D
BG, KH = 4, 2                 # batch groups x kout halves
BPC = B // BG                 # 1024 batch rows per core
KHC = K_OUT // KH             # 1024 kout cols per core
KPC = K_OUT // N_CORES        # 256 diag rows per core
FT = FEAT // 128              # 32 feature tiles
DT = D // 128                 # 16 e1-space feature tiles
KTL = KHC // 128              # 8 local kout tiles
KGROUPS = (3, 3, 2)           # kout tile groups (2*g PSUM banks each)
DIAG_SCALE = 0.5 / (B * D)    # 0.5: the 8-core allreduce double-counts rows

_CACHE = {}


def _build_nc():
    import concourse.bacc as bacc
    import concourse.tile as tile
    import concourse.mybir as mybir
    from concourse.tile_rust import add_dep_helper

    dt = mybir.dt
    nc = bacc.Bacc("TRN2", target_bir_lowering=False, debug=False,
                   num_devices=N_CORES)

    xt = nc.dram_tensor("xt", [FEAT, BPC], dt.bfloat16, kind="ExternalInput").ap()
    v = nc.dram_tensor("v", [FEAT, KHC], dt.bfloat16, kind="ExternalInput").ap()
    wt = nc.dram_tensor("wt", [D, KPC], dt.float32r, kind="ExternalInput").ap()
    bvec = nc.dram_tensor("bvec", [1, KPC], dt.float32, kind="ExternalInput").ap()
    out = nc.dram_tensor("out", [KHC, BPC], dt.float32, kind="ExternalOutput").ap()

    core_ids = list(range(N_CORES))
    ag_groups = [[0, 2, 4, 6], [1, 3, 5, 7]]

    with tile.TileContext(nc) as tc:
        with tc.tile_pool(name="xpool", bufs=1) as xpool, \
             tc.tile_pool(name="vpool", bufs=1) as vpool, \
             tc.tile_pool(name="wpool", bufs=1) as wpool, \
             tc.tile_pool(name="spool", bufs=1) as spool, \
             tc.tile_pool(name="scratch", bufs=2) as scratch, \
             tc.tile_pool(name="stage", bufs=1) as stage_pool, \
             tc.tile_pool(name="opool", bufs=2) as opool, \
             tc.tile_pool(name="psum", bufs=6, space="PSUM") as pp, \
             tc.tile_pool(name="psd", bufs=1, space="PSUM") as ppd, \
             tc.tile_pool(name="dram", bufs=1, space="DRAM") as dram:

            # ---- interleaved resident loads: x tiles j & 16+j, v tiles ----
            x_all = xpool.tile([128, FT * BPC], dt.bfloat16)
            v_all = vpool.tile([128, FT * KHC], dt.bfloat16)
            for j in range(DT):
                for jj in (j, DT + j):
                    nc.sync.dma_start(x_all[:, jj * BPC:(jj + 1) * BPC],
                                      xt[jj * 128:(jj + 1) * 128, :])
                for jj in (j, DT + j):
                    nc.sync.dma_start(v_all[:, jj * KHC:(jj + 1) * KHC],
                                      v[jj * 128:(jj + 1) * 128, :])
            # diag-path weights (needed only ~mid-kernel; end of DMA queue)
            wt_sb = wpool.tile([128, DT * KPC], dt.float32r)
            for j in range(DT):
                nc.sync.dma_start(wt_sb[:, j * KPC:(j + 1) * KPC],
                                  wt[j * 128:(j + 1) * 128, :])
            b_sb = spool.tile([1, KPC], dt.float32, name="b_sb")
            nc.sync.dma_start(b_sb[:], bvec[:])

            # ---- diag partials on DVE as x tiles arrive ----
            s_sb = spool.tile([128, DT], dt.float32)
            for j in range(DT):
                prod = scratch.tile([128, BPC], dt.bfloat16, tag="prod",
                                    name=f"prod{j}")
                nc.vector.tensor_mul(
                    prod[:],
                    x_all[:, j * BPC:(j + 1) * BPC],
                    x_all[:, (DT + j) * BPC:(DT + j + 1) * BPC])
                nc.vector.tensor_reduce(s_sb[:, j:j + 1], prod[:],
                                        mybir.AxisListType.X,
                                        mybir.AluOpType.add)

            # ---- AllReduce s over all cores (8 KiB) ----
            s_in = dram.tile([128, DT], dt.float32)
            s_out = dram.tile([128, DT], dt.float32, addr_space="Shared")
            nc.sync.dma_start(s_in[:], s_sb[:])
            nc.gpsimd.collective_compute(
                "AllReduce", mybir.AluOpType.add,
                replica_groups=[core_ids],
                ins=[s_in.opt()], outs=[s_out.opt()])
            s_r = spool.tile([128, DT], dt.float32, name="s_r")
            nc.sync.dma_start(s_r[:], s_out[:])

            # ---- main matmul: out^T = V_half^T @ x^T, bf16 on TensorE ----
            stage = stage_pool.tile([128, 6 * BPC], dt.float32, name="stage")
            diag_cols = spool.tile([128, KTL], dt.float32, name="diag_cols")
            diag_mm_first = None
            g1_last_mm = None
            k0 = 0
            for kg, g in enumerate(KGROUPS):
                pss = [[pp.tile([128, 512], dt.float32, tag="ps",
                                name=f"ps{kg}_{q}_{b2}")
                        for b2 in range(2)] for q in range(g)]
                for j in range(FT):
                    for q in range(g):
                        for b2 in range(2):
                            mm = nc.tensor.matmul(
                                pss[q][b2][:],
                                v_all[:, j * KHC + (k0 + q) * 128:
                                      j * KHC + (k0 + q + 1) * 128],
                                x_all[:, j * BPC + b2 * 512:
                                      j * BPC + (b2 + 1) * 512],
                                start=(j == 0), stop=(j == FT - 1))
                            if kg == 1:
                                g1_last_mm = mm
                if kg < 2:
                    # drain PSUM -> stage, banks split across DVE/ScalarE
                    for q in range(g):
                        kt = k0 + q
                        for b2 in range(2):
                            dst = stage[:, kt * BPC + b2 * 512:
                                        kt * BPC + (b2 + 1) * 512]
                            if b2 == 0:
                                nc.vector.tensor_copy(dst, pss[q][b2][:])
                            else:
                                nc.scalar.activation(
                                    dst, pss[q][b2][:],
                                    mybir.ActivationFunctionType.Copy)
                else:
                    # last group: tanh straight out of PSUM (ScalarE)
                    for q in range(g):
                        kt = k0 + q
                        ot2 = opool.tile([128, BPC], dt.float32, tag="ot",
                                         name=f"ot_g2_{q}")
                        for b2 in range(2):
                            nc.scalar.activation(
                                ot2[:, b2 * 512:(b2 + 1) * 512],
                                pss[q][b2][:],
                                mybir.ActivationFunctionType.Tanh,
                                bias=diag_cols[:, kt:kt + 1])
                        nc.sync.dma_start(out[kt * 128:(kt + 1) * 128, :],
                                          ot2[:])
                k0 += g

                if kg == 1:
                    # ---- diag slice: [1, KPC] = s @ wt, f32r matmuls ----
                    # (pinned here: AllReduce is finished long before, and
                    # the main matmul stream is never blocked by it)
                    ps_d = ppd.tile([1, KPC], dt.float32)
                    for j in range(DT):
                        mm = nc.tensor.matmul(
                            ps_d[:],
                            s_r[:, j:j + 1].bitcast(dt.float32r),
                            wt_sb[:, j * KPC:(j + 1) * KPC],
                            start=(j == 0), stop=(j == DT - 1))
                        if diag_mm_first is None:
                            diag_mm_first = mm
                            add_dep_helper(g1_last_mm.ins, mm.ins, sync=False,
                                           reason="diag mms after main g1")
                    diag_sb = spool.tile([1, KPC], dt.float32, name="diag_sb")
                    nc.vector.tensor_scalar_mul(diag_sb[:], ps_d[:], DIAG_SCALE)
                    nc.vector.tensor_add(diag_sb[:], diag_sb[:], b_sb[:])

                    # ---- AllGather diag within kout-half subgroup (1 KiB) ----
                    d_in = dram.tile([1, KPC], dt.float32, name="d_in")
                    d_out = dram.tile([KTL, 128], dt.float32, name="d_out")
                    nc.sync.dma_start(d_in[:], diag_sb[:])
                    nc.gpsimd.collective_compute(
                        "AllGather", mybir.AluOpType.bypass,
                        replica_groups=ag_groups,
                        ins=[d_in.opt()], outs=[d_out.opt()])
                    # [128, KTL]: partition p, col k <- diag_half[k*128 + p]
                    nc.sync.dma_start(diag_cols[:],
                                      d_out[:].rearrange("k p -> p k"))

                    # ---- tanh for staged groups 0/1 ----
                    for kt in range(6):
                        ot = opool.tile([128, BPC], dt.float32, tag="ot",
                                        name=f"ot{kt}")
                        nc.scalar.activation(
                            ot[:], stage[:, kt * BPC:(kt + 1) * BPC],
                            mybir.ActivationFunctionType.Tanh,
                            bias=diag_cols[:, kt:kt + 1])
                        nc.sync.dma_start(out[kt * 128:(kt + 1) * 128, :],
                                          ot[:])

    nc.compile()
    return nc


def _get_nc():
    if "nc" not in _CACHE:
        _CACHE["nc"] = _build_nc()
    return _CACHE["nc"]


def make_in_maps(e1, e2, W, V, b):
    import ml_dtypes
    bf16 = ml_dtypes.bfloat16

    in_maps = []
    for c in range(N_CORES):
        g, h = c // 2, c % 2
        sc = h * 4 + g            # permuted diag-slice index (see module doc)
        rows = slice(g * BPC, (g + 1) * BPC)
        krows = slice(sc * KPC, (sc + 1) * KPC)
        xt = np.ascontiguousarray(
            np.concatenate([e1[rows], e2[rows]], axis=1).T).astype(bf16)
        in_maps.append({
            "xt": xt,
            "v": np.ascontiguousarray(V[:, h * KHC:(h + 1) * KHC]).astype(bf16),
            "wt": np.ascontiguousarray(W[krows].T),
            "bvec": b[krows].reshape(1, KPC),
        })
    return in_maps


def kernel(e1, e2, W, V, b):
    from concourse.bass_utils import run_bass_kernel_spmd

    e1 = np.asarray(e1, dtype=np.float32)
    e2 = np.asarray(e2, dtype=np.float32)
    W = np.asarray(W, dtype=np.float32)
    V = np.asarray(V, dtype=np.float32)
    b = np.asarray(b, dtype=np.float32)

    nc = _get_nc()
    res = run_bass_kernel_spmd(nc, make_in_maps(e1, e2, W, V, b),
                               list(range(N_CORES)))
    out = np.empty((B, K_OUT), dtype=np.float32)
    for c in range(N_CORES):
        g, h = c // 2, c % 2
        out[g * BPC:(g + 1) * BPC, h * KHC:(h + 1) * KHC] = res.results[c]["out"].T
    return out


# revision 8
# speedup vs baseline: 1.0051x; 1.0051x over previous
"""Trainium2 Bass kernel for nn_NeuralTensorDiagLayer.

Computes out = tanh(concat([e1, e2], -1) @ V + diag + b) where
diag[k] = (sum_b(e1*e2) @ W[k]) / (B*D), broadcast over batch.

Sharding (8 NeuronCores, 2D: 4 batch groups x 2 k_out halves):
  - Core c handles batch rows [1024*(c//2), 1024*(c//2+1)) and k_out
    columns [1024*(c%2), 1024*(c%2+1)).
  - All main-path tensors are cast to bf16 on the host (V from
    uniform(-1,1), x = concat(e1,e2) transposed): rel-err budget is 2e-2
    and bf16 end-to-end measures ~1e-2, while halving HBM traffic and
    keeping the TensorEngine at 1 col/cycle.
  - x^T and V stream into SBUF fully resident via interleaved [128,1024]
    DMAs (2 KiB lines) ordered so contraction tile j (x1_j, x2_j, v_j,
    v_{16+j}) lands early; the main matmul's first PSUM group chases the
    DMA stream and the rest runs from SBUF at full rate.
  - Main matmul: 3 PSUM groups of (3,3,2) k-tiles x 2 batch-half banks.
    Groups 0/1 drain PSUM->stage with DVE/ScalarE copies split per bank;
    group 2 is tanh'ed directly out of PSUM (ScalarE reads PSUM).
  - diag: per-tile fused mul+reduce partials on DVE as x tiles arrive,
    8-core AllReduce of s=[128,16] (each batch row counted twice -> 0.5
    folded into DIAG_SCALE), then a 256-col diag slice as 16 f32r
    matmuls (N=256 -> 1 cycle/row) pinned AFTER main group 1 in the
    TensorE stream (AllReduce is long done by then; pinning avoids the
    baseline's 17us TensorE stall), AllGather over [[0,2,4,6],[1,3,5,7]]
    assembles each k_out half (diag slice index sc = (c%2)*4 + c//2 is
    applied host-side so the device program stays SPMD-identical).
  - tanh+bias on ScalarE with diag as per-partition bias, fp32 out tiles
    DMA'd per k-tile ([k_out, batch] transposed); host reassembles.
"""

import os
import sys

for _p in ("/opt/trn_rl_repo", "/root/.axon_site/_ro/trn_rl_repo"):
    if os.path.isdir(_p) and _p not in sys.path:
        sys.path.append(_p)

import numpy as np

N_CORES = 8
B, D, K_OUT = 4096, 2048, 2048
FEAT = 2 * D
BG, KH = 4, 2                 # batch groups x kout halves
BPC = B // BG                 # 1024 batch rows per core
KHC = K_OUT // KH             # 1024 kout cols per core
KPC = K_OUT // N_CORES        # 256 diag rows per core
FT = FEAT // 128              # 32 feature tiles
DT = D // 128                 # 16 e1-space feature tiles
KTL = KHC // 128              # 8 local kout tiles
KGROUPS = (4, 4)              # kout tile groups (2*g PSUM banks each)
DIAG_SCALE = 0.5 / (B * D)    # 0.5: the 8-core allreduce double-counts rows

_CACHE = {}


def _build_nc():
    import concourse.bacc as bacc
    import concourse.tile as tile
    import concourse.mybir as mybir
    from concourse.tile_rust import add_dep_helper

    dt = mybir.dt
    nc = bacc.Bacc("TRN2", target_bir_lowering=False, debug=False,
                   num_devices=N_CORES)

    xt = nc.dram_tensor("xt", [FEAT, BPC], dt.bfloat16, kind="ExternalInput").ap()
    v = nc.dram_tensor("v", [FEAT, KHC], dt.bfloat16, kind="ExternalInput").ap()
    wt = nc.dram_tensor("wt", [D, KPC], dt.float32r, kind="ExternalInput").ap()
    bvec = nc.dram_tensor("bvec", [1, KPC], dt.float32, kind="ExternalInput").ap()
    out = nc.dram_tensor("out", [KHC, BPC], dt.float32, kind="ExternalOutput").ap()

    core_ids = list(range(N_CORES))
    ag_groups = [[0, 2, 4, 6], [1, 3, 5, 7]]

    with tile.TileContext(nc) as tc:
        with tc.tile_pool(name="xpool", bufs=1) as xpool, \
             tc.tile_pool(name="vpool", bufs=1) as vpool, \
             tc.tile_pool(name="wpool", bufs=1) as wpool, \
             tc.tile_pool(name="spool", bufs=1) as spool, \
             tc.tile_pool(name="scratch", bufs=2) as scratch, \
             tc.tile_pool(name="stage", bufs=1) as stage_pool, \
             tc.tile_pool(name="opool", bufs=2) as opool, \
             tc.tile_pool(name="psum", bufs=6, space="PSUM") as pp, \
             tc.tile_pool(name="psd", bufs=1, space="PSUM") as ppd, \
             tc.tile_pool(name="dram", bufs=1, space="DRAM") as dram:

            # ---- interleaved resident loads, 1 MiB quad transfers ----
            # quad i covers 4 consecutive 128-row tiles as [128, 4*cols] via
            # a DRAM rearrange; order (x1_qi, x2_qi, v_q2i, v_q2i+1) so the
            # main matmul's j-order consumption chases the DMA stream.
            x_all = xpool.tile([128, FT * BPC], dt.bfloat16)
            v_all = vpool.tile([128, FT * KHC], dt.bfloat16)

            def quad_load(dst_tile, dst_cols, src, tile0):
                nc.sync.dma_start(
                    dst_tile[:, tile0 * dst_cols:(tile0 + 4) * dst_cols]
                    .rearrange("p (j c) -> p j c", j=4),
                    src[tile0 * 128:(tile0 + 4) * 128, :]
                    .rearrange("(j p) c -> p j c", p=128))

            for i in range(4):
                quad_load(x_all, BPC, xt, 4 * i)            # x1 tiles 4i..4i+3
                quad_load(x_all, BPC, xt, DT + 4 * i)       # x2 tiles
                quad_load(v_all, KHC, v, 8 * i)             # v tiles 8i..8i+3
                quad_load(v_all, KHC, v, 8 * i + 4)         # v tiles 8i+4..8i+7
            # diag-path weights (needed only ~mid-kernel; end of DMA queue)
            wt_sb = wpool.tile([128, DT * KPC], dt.float32r)
            nc.sync.dma_start(wt_sb[:].rearrange("p (j c) -> p j c", j=DT),
                              wt[:].rearrange("(j p) c -> p j c", p=128))
            b_sb = spool.tile([1, KPC], dt.float32, name="b_sb")
            nc.sync.dma_start(b_sb[:], bvec[:])

            # ---- diag partials on DVE as x tiles arrive ----
            s_sb = spool.tile([128, DT], dt.float32)
            for j in range(DT):
                prod = scratch.tile([128, BPC], dt.bfloat16, tag="prod",
                                    name=f"prod{j}")
                nc.vector.tensor_mul(
                    prod[:],
                    x_all[:, j * BPC:(j + 1) * BPC],
                    x_all[:, (DT + j) * BPC:(DT + j + 1) * BPC])
                nc.vector.tensor_reduce(s_sb[:, j:j + 1], prod[:],
                                        mybir.AxisListType.X,
                                        mybir.AluOpType.add)

            # ---- AllReduce s over all cores (8 KiB) ----
            s_in = dram.tile([128, DT], dt.float32)
            s_out = dram.tile([128, DT], dt.float32, addr_space="Shared")
            nc.sync.dma_start(s_in[:], s_sb[:])
            nc.gpsimd.collective_compute(
                "AllReduce", mybir.AluOpType.add,
                replica_groups=[core_ids],
                ins=[s_in.opt()], outs=[s_out.opt()])
            s_r = spool.tile([128, DT], dt.float32, name="s_r")
            nc.sync.dma_start(s_r[:], s_out[:])

            # ---- main matmul: out^T = V_half^T @ x^T, bf16 on TensorE ----
            n_staged = KGROUPS[0]
            stage = stage_pool.tile([128, n_staged * BPC], dt.float32,
                                    name="stage")
            diag_cols = spool.tile([128, KTL], dt.float32, name="diag_cols")
            diag_mm_first = None
            g0_last_mm = None
            k0 = 0
            for kg, g in enumerate(KGROUPS):
                pss = [[pp.tile([128, 512], dt.float32, tag="ps",
                                name=f"ps{kg}_{q}_{b2}")
                        for b2 in range(2)] for q in range(g)]
                for j in range(FT):
                    for q in range(g):
                        for b2 in range(2):
                            mm = nc.tensor.matmul(
                                pss[q][b2][:],
                                v_all[:, j * KHC + (k0 + q) * 128:
                                      j * KHC + (k0 + q + 1) * 128],
                                x_all[:, j * BPC + b2 * 512:
                                      j * BPC + (b2 + 1) * 512],
                                start=(j == 0), stop=(j == FT - 1))
                            if kg == 0:
                                g0_last_mm = mm
                if kg == 0:
                    # drain PSUM -> stage, banks split across DVE/ScalarE
                    for q in range(g):
                        kt = k0 + q
                        for b2 in range(2):
                            dst = stage[:, kt * BPC + b2 * 512:
                                        kt * BPC + (b2 + 1) * 512]
                            if b2 == 0:
                                nc.vector.tensor_copy(dst, pss[q][b2][:])
                            else:
                                nc.scalar.activation(
                                    dst, pss[q][b2][:],
                                    mybir.ActivationFunctionType.Copy)
                else:
                    # last group: tanh straight out of PSUM (ScalarE)
                    for q in range(g):
                        kt = k0 + q
                        ot2 = opool.tile([128, BPC], dt.float32, tag="ot",
                                         name=f"ot_g1_{q}")
                        for b2 in range(2):
                            nc.scalar.activation(
                                ot2[:, b2 * 512:(b2 + 1) * 512],
                                pss[q][b2][:],
                                mybir.ActivationFunctionType.Tanh,
                                bias=diag_cols[:, kt:kt + 1])
                        nc.sync.dma_start(out[kt * 128:(kt + 1) * 128, :],
                                          ot2[:])
                k0 += g

                if kg == 0:
                    # ---- diag slice: [1, KPC] = s @ wt, f32r matmuls ----
                    # (pinned between the groups: AllReduce is finished well
                    # before group 0's matmuls are, so the stream never
                    # blocks on the collective)
                    ps_d = ppd.tile([1, KPC], dt.float32)
                    for j in range(DT):
                        mm = nc.tensor.matmul(
                            ps_d[:],
                            s_r[:, j:j + 1].bitcast(dt.float32r),
                            wt_sb[:, j * KPC:(j + 1) * KPC],
                            start=(j == 0), stop=(j == DT - 1))
                        if diag_mm_first is None:
                            diag_mm_first = mm
                            add_dep_helper(g0_last_mm.ins, mm.ins, sync=False,
                                           reason="diag mms after main g0")
                    diag_sb = spool.tile([1, KPC], dt.float32, name="diag_sb")
                    nc.vector.tensor_scalar_mul(diag_sb[:], ps_d[:], DIAG_SCALE)
                    nc.vector.tensor_add(diag_sb[:], diag_sb[:], b_sb[:])

                    # ---- AllGather diag within kout-half subgroup (1 KiB) ----
                    d_in = dram.tile([1, KPC], dt.float32, name="d_in")
                    d_out = dram.tile([KTL, 128], dt.float32, name="d_out")
                    nc.sync.dma_start(d_in[:], diag_sb[:])
                    nc.gpsimd.collective_compute(
                        "AllGather", mybir.AluOpType.bypass,
                        replica_groups=ag_groups,
                        ins=[d_in.opt()], outs=[d_out.opt()])
                    # [128, KTL]: partition p, col k <- diag_half[k*128 + p]
                    nc.sync.dma_start(diag_cols[:],
                                      d_out[:].rearrange("k p -> p k"))

                    # ---- tanh for staged group 0 (overlaps group 1) ----
                    for kt in range(n_staged):
                        ot = opool.tile([128, BPC], dt.float32, tag="ot",
                                        name=f"ot{kt}")
                        nc.scalar.activation(
                            ot[:], stage[:, kt * BPC:(kt + 1) * BPC],
                            mybir.ActivationFunctionType.Tanh,
                            bias=diag_cols[:, kt:kt + 1])
                        nc.sync.dma_start(out[kt * 128:(kt + 1) * 128, :],
                                          ot[:])

    nc.compile()
    return nc


def _get_nc():
    if "nc" not in _CACHE:
        _CACHE["nc"] = _build_nc()
    return _CACHE["nc"]


def make_in_maps(e1, e2, W, V, b):
    import ml_dtypes
    bf16 = ml_dtypes.bfloat16

    in_maps = []
    for c in range(N_CORES):
        g, h = c // 2, c % 2
        sc = h * 4 + g            # permuted diag-slice index (see module doc)
        rows = slice(g * BPC, (g + 1) * BPC)
        krows = slice(sc * KPC, (sc + 1) * KPC)
        xt = np.ascontiguousarray(
            np.concatenate([e1[rows], e2[rows]], axis=1).T).astype(bf16)
        in_maps.append({
            "xt": xt,
            "v": np.ascontiguousarray(V[:, h * KHC:(h + 1) * KHC]).astype(bf16),
            "wt": np.ascontiguousarray(W[krows].T),
            "bvec": b[krows].reshape(1, KPC),
        })
    return in_maps


def kernel(e1, e2, W, V, b):
    from concourse.bass_utils import run_bass_kernel_spmd

    e1 = np.asarray(e1, dtype=np.float32)
    e2 = np.asarray(e2, dtype=np.float32)
    W = np.asarray(W, dtype=np.float32)
    V = np.asarray(V, dtype=np.float32)
    b = np.asarray(b, dtype=np.float32)

    nc = _get_nc()
    res = run_bass_kernel_spmd(nc, make_in_maps(e1, e2, W, V, b),
                               list(range(N_CORES)))
    out = np.empty((B, K_OUT), dtype=np.float32)
    for c in range(N_CORES):
        g, h = c // 2, c % 2
        out[g * BPC:(g + 1) * BPC, h * KHC:(h + 1) * KHC] = res.results[c]["out"].T
    return out


# revision 13
# speedup vs baseline: 1.0788x; 1.0732x over previous
"""Trainium2 Bass kernel for nn_NeuralTensorDiagLayer.

Computes out = tanh(concat([e1, e2], -1) @ V + diag + b) where
diag[k] = (sum_b(e1*e2) @ W[k]) / (B*D), broadcast over batch.

Sharding (8 NeuronCores, 2D: 4 batch groups x 2 k_out halves):
  - Core c handles batch rows [1024*(c//2), 1024*(c//2+1)) and k_out
    columns [1024*(c%2), 1024*(c%2+1)).
  - All main-path tensors are cast to bf16 on the host (V from
    uniform(-1,1), x = concat(e1,e2) transposed): rel-err budget is 2e-2
    and bf16 end-to-end measures ~1e-2, while halving HBM traffic and
    keeping the TensorEngine at 1 col/cycle.
  - x^T and V stream into SBUF fully resident via interleaved [128,1024]
    DMAs (2 KiB lines) ordered so contraction tile j (x1_j, x2_j, v_j,
    v_{16+j}) lands early; the main matmul's first PSUM group chases the
    DMA stream and the rest runs from SBUF at full rate.
  - Main matmul: 3 PSUM groups of (3,3,2) k-tiles x 2 batch-half banks.
    Groups 0/1 drain PSUM->stage with DVE/ScalarE copies split per bank;
    group 2 is tanh'ed directly out of PSUM (ScalarE reads PSUM).
  - diag: per-tile fused mul+reduce partials on DVE as x tiles arrive,
    8-core AllReduce of s=[128,16] (each batch row counted twice -> 0.5
    folded into DIAG_SCALE), then a 256-col diag slice as 16 f32r
    matmuls (N=256 -> 1 cycle/row) pinned AFTER main group 1 in the
    TensorE stream (AllReduce is long done by then; pinning avoids the
    baseline's 17us TensorE stall), AllGather over [[0,2,4,6],[1,3,5,7]]
    assembles each k_out half (diag slice index sc = (c%2)*4 + c//2 is
    applied host-side so the device program stays SPMD-identical).
  - tanh+bias on ScalarE with diag as per-partition bias, fp32 out tiles
    DMA'd per k-tile ([k_out, batch] transposed); host reassembles.
"""

import os
import sys

for _p in ("/opt/trn_rl_repo", "/root/.axon_site/_ro/trn_rl_repo"):
    if os.path.isdir(_p) and _p not in sys.path:
        sys.path.append(_p)

import numpy as np

N_CORES = 8
B, D, K_OUT = 4096, 2048, 2048
FEAT = 2 * D
BG, KH = 4, 2                 # batch groups x kout halves
BPC = B // BG                 # 1024 batch rows per core
KHC = K_OUT // KH             # 1024 kout cols per core
KPC = K_OUT // N_CORES        # 256 diag rows per core
FT = FEAT // 128              # 32 feature tiles
DT = D // 128                 # 16 e1-space feature tiles
KTL = KHC // 128              # 8 local kout tiles
KGROUPS = (4, 4)              # kout tile groups (2*g PSUM banks each)
DIAG_SCALE = 0.5 / (B * D)    # 0.5: the 8-core allreduce double-counts rows

_CACHE = {}


def _build_nc():
    import concourse.bacc as bacc
    import concourse.tile as tile
    import concourse.mybir as mybir
    from concourse.tile_rust import add_dep_helper

    dt = mybir.dt
    nc = bacc.Bacc("TRN2", target_bir_lowering=False, debug=False,
                   num_devices=N_CORES)

    xt = nc.dram_tensor("xt", [FEAT, BPC], dt.bfloat16, kind="ExternalInput").ap()
    v = nc.dram_tensor("v", [FEAT, KHC], dt.bfloat16, kind="ExternalInput").ap()
    wt = nc.dram_tensor("wt", [D, KPC], dt.bfloat16, kind="ExternalInput").ap()
    bvec = nc.dram_tensor("bvec", [1, KPC], dt.float32, kind="ExternalInput").ap()
    out = nc.dram_tensor("out", [KHC, BPC], dt.bfloat16, kind="ExternalOutput").ap()

    core_ids = list(range(N_CORES))
    ag_groups = [[0, 2, 4, 6], [1, 3, 5, 7]]

    with tile.TileContext(nc) as tc:
        with tc.tile_pool(name="xpool", bufs=1) as xpool, \
             tc.tile_pool(name="vpool", bufs=1) as vpool, \
             tc.tile_pool(name="wpool", bufs=1) as wpool, \
             tc.tile_pool(name="spool", bufs=1) as spool, \
             tc.tile_pool(name="scratch", bufs=2) as scratch, \
             tc.tile_pool(name="stage", bufs=1) as stage_pool, \
             tc.tile_pool(name="opool", bufs=2) as opool, \
             tc.tile_pool(name="psum", bufs=6, space="PSUM") as pp, \
             tc.tile_pool(name="psd", bufs=1, space="PSUM") as ppd, \
             tc.tile_pool(name="dram", bufs=1, space="DRAM") as dram:

            # ---- interleaved resident loads, 1 MiB quad transfers ----
            # quad i covers 4 consecutive 128-row tiles as [128, 4*cols] via
            # a DRAM rearrange; order (x1_qi, x2_qi, v_q2i, v_q2i+1) so the
            # main matmul's j-order consumption chases the DMA stream.
            x_all = xpool.tile([128, FT * BPC], dt.bfloat16)
            v_all = vpool.tile([128, FT * KHC], dt.bfloat16)

            def multi_load(dst_tile, dst_cols, src, tile0, n):
                nc.sync.dma_start(
                    dst_tile[:, tile0 * dst_cols:(tile0 + n) * dst_cols]
                    .rearrange("p (j c) -> p j c", j=n),
                    src[tile0 * 128:(tile0 + n) * 128, :]
                    .rearrange("(j p) c -> p j c", p=128))

            # all HWDGE DMAs drain one FIFO queue in issue order, so issue
            # exactly in the main loop's consumption order (j-block b needs
            # x tiles 4b.. and v tiles 4b..); block 0 split in pairs so the
            # first matmul can start ~3us earlier.
            multi_load(x_all, BPC, xt, 0, 2)
            multi_load(v_all, KHC, v, 0, 2)
            multi_load(x_all, BPC, xt, 2, 2)
            multi_load(v_all, KHC, v, 2, 2)
            for b in range(1, 8):
                multi_load(x_all, BPC, xt, 4 * b, 4)
                multi_load(v_all, KHC, v, 4 * b, 4)
            # diag-path weights (needed only ~mid-kernel; end of DMA queue)
            wt_sb = wpool.tile([128, DT * KPC], dt.bfloat16)
            multi_load(wt_sb, KPC, wt, 0, DT)
            b_sb = spool.tile([1, KPC], dt.float32, name="b_sb")
            nc.sync.dma_start(b_sb[:], bvec[:])

            # ---- diag partials on DVE as x tiles arrive ----
            s_sb = spool.tile([128, DT], dt.float32)
            for j in range(DT):
                prod = scratch.tile([128, BPC], dt.bfloat16, tag="prod",
                                    name=f"prod{j}")
                nc.vector.tensor_mul(
                    prod[:],
                    x_all[:, j * BPC:(j + 1) * BPC],
                    x_all[:, (DT + j) * BPC:(DT + j + 1) * BPC])
                nc.vector.tensor_reduce(s_sb[:, j:j + 1], prod[:],
                                        mybir.AxisListType.X,
                                        mybir.AluOpType.add)

            # ---- AllReduce s over all cores (8 KiB) ----
            s_in = dram.tile([128, DT], dt.float32)
            s_out = dram.tile([128, DT], dt.float32, addr_space="Shared")
            nc.sync.dma_start(s_in[:], s_sb[:])
            nc.gpsimd.collective_compute(
                "AllReduce", mybir.AluOpType.add,
                replica_groups=[core_ids],
                ins=[s_in.opt()], outs=[s_out.opt()])
            s_r = spool.tile([128, DT], dt.float32, name="s_r")
            nc.sync.dma_start(s_r[:], s_out[:])

            # ---- main matmul: out^T = V_half^T @ x^T, bf16 on TensorE ----
            n_staged = KGROUPS[0]
            stage = stage_pool.tile([128, n_staged * BPC], dt.float32,
                                    name="stage")
            diag_cols = spool.tile([128, KTL], dt.float32, name="diag_cols")
            diag_mm_first = None
            g0_last_mm = None
            k0 = 0
            for kg, g in enumerate(KGROUPS):
                pss = [[pp.tile([128, 512], dt.float32, tag="ps",
                                name=f"ps{kg}_{q}_{b2}")
                        for b2 in range(2)] for q in range(g)]
                for j in range(FT):
                    for q in range(g):
                        for b2 in range(2):
                            mm = nc.tensor.matmul(
                                pss[q][b2][:],
                                v_all[:, j * KHC + (k0 + q) * 128:
                                      j * KHC + (k0 + q + 1) * 128],
                                x_all[:, j * BPC + b2 * 512:
                                      j * BPC + (b2 + 1) * 512],
                                start=(j == 0), stop=(j == FT - 1))
                            if kg == 0:
                                g0_last_mm = mm
                if kg == 0:
                    # drain PSUM -> stage, banks split across DVE/ScalarE
                    for q in range(g):
                        kt = k0 + q
                        for b2 in range(2):
                            dst = stage[:, kt * BPC + b2 * 512:
                                        kt * BPC + (b2 + 1) * 512]
                            if b2 == 0:
                                nc.vector.tensor_copy(dst, pss[q][b2][:])
                            else:
                                nc.scalar.activation(
                                    dst, pss[q][b2][:],
                                    mybir.ActivationFunctionType.Copy)
                else:
                    # last group: tanh straight out of PSUM (ScalarE)
                    for q in range(g):
                        kt = k0 + q
                        ot2 = opool.tile([128, BPC], dt.bfloat16, tag="ot",
                                         name=f"ot_g1_{q}")
                        for b2 in range(2):
                            nc.scalar.activation(
                                ot2[:, b2 * 512:(b2 + 1) * 512],
                                pss[q][b2][:],
                                mybir.ActivationFunctionType.Tanh,
                                bias=diag_cols[:, kt:kt + 1])
                        nc.sync.dma_start(out[kt * 128:(kt + 1) * 128, :],
                                          ot2[:])
                k0 += g

                if kg == 0:
                    # ---- diag slice: [1, KPC] = s @ wt, bf16 matmuls ----
                    # (pinned between the groups: AllReduce is finished well
                    # before group 0's matmuls are, so the stream never
                    # blocks on the collective)
                    s_bf = spool.tile([128, DT], dt.bfloat16, name="s_bf")
                    nc.vector.tensor_copy(s_bf[:], s_r[:])
                    ps_d = ppd.tile([1, KPC], dt.float32)
                    for j in range(DT):
                        mm = nc.tensor.matmul(
                            ps_d[:],
                            s_bf[:, j:j + 1],
                            wt_sb[:, j * KPC:(j + 1) * KPC],
                            start=(j == 0), stop=(j == DT - 1))
                        if diag_mm_first is None:
                            diag_mm_first = mm
                            add_dep_helper(g0_last_mm.ins, mm.ins, sync=False,
                                           reason="diag mms after main g0")
                    diag_sb = spool.tile([1, KPC], dt.float32, name="diag_sb")
                    nc.vector.tensor_scalar_mul(diag_sb[:], ps_d[:], DIAG_SCALE)
                    nc.vector.tensor_add(diag_sb[:], diag_sb[:], b_sb[:])

                    # ---- AllGather diag within kout-half subgroup (1 KiB) ----
                    d_in = dram.tile([1, KPC], dt.float32, name="d_in")
                    d_out = dram.tile([KTL, 128], dt.float32, name="d_out")
                    nc.sync.dma_start(d_in[:], diag_sb[:])
                    nc.gpsimd.collective_compute(
                        "AllGather", mybir.AluOpType.bypass,
                        replica_groups=ag_groups,
                        ins=[d_in.opt()], outs=[d_out.opt()])
                    # [128, KTL]: partition p, col k <- diag_half[k*128 + p]
                    nc.sync.dma_start(diag_cols[:],
                                      d_out[:].rearrange("k p -> p k"))

                    # ---- tanh for staged group 0 (overlaps group 1) ----
                    for kt in range(n_staged):
                        ot = opool.tile([128, BPC], dt.bfloat16, tag="ot",
                                        name=f"ot{kt}")
                        nc.scalar.activation(
                            ot[:], stage[:, kt * BPC:(kt + 1) * BPC],
                            mybir.ActivationFunctionType.Tanh,
                            bias=diag_cols[:, kt:kt + 1])
                        nc.sync.dma_start(out[kt * 128:(kt + 1) * 128, :],
                                          ot[:])

    nc.compile()
    return nc


def _get_nc():
    if "nc" not in _CACHE:
        _CACHE["nc"] = _build_nc()
    return _CACHE["nc"]


def make_in_maps(e1, e2, W, V, b):
    import ml_dtypes
    bf16 = ml_dtypes.bfloat16

    in_maps = []
    for c in range(N_CORES):
        g, h = c // 2, c % 2
        sc = h * 4 + g            # permuted diag-slice index (see module doc)
        rows = slice(g * BPC, (g + 1) * BPC)
        krows = slice(sc * KPC, (sc + 1) * KPC)
        xt = np.ascontiguousarray(
            np.concatenate([e1[rows], e2[rows]], axis=1).T).astype(bf16)
        in_maps.append({
            "xt": xt,
            "v": np.ascontiguousarray(V[:, h * KHC:(h + 1) * KHC]).astype(bf16),
            "wt": np.ascontiguousarray(W[krows].T).astype(bf16),
            "bvec": b[krows].reshape(1, KPC),
        })
    return in_maps


def kernel(e1, e2, W, V, b):
    from concourse.bass_utils import run_bass_kernel_spmd

    e1 = np.asarray(e1, dtype=np.float32)
    e2 = np.asarray(e2, dtype=np.float32)
    W = np.asarray(W, dtype=np.float32)
    V = np.asarray(V, dtype=np.float32)
    b = np.asarray(b, dtype=np.float32)

    nc = _get_nc()
    res = run_bass_kernel_spmd(nc, make_in_maps(e1, e2, W, V, b),
                               list(range(N_CORES)))
    out = np.empty((B, K_OUT), dtype=np.float32)
    for c in range(N_CORES):
        g, h = c // 2, c % 2
        out[g * BPC:(g + 1) * BPC, h * KHC:(h + 1) * KHC] = \
            res.results[c]["out"].T.astype(np.float32)
    return out
